# revision 13
# baseline (speedup 1.0000x reference)
"""AdEx neuron simulation kernel for 8 Trainium2 NeuronCores.

Serial per-step chain in the Y = V - V_reset state (w stays 0 for a=b=0):
    e_t = exp(s*Y_{t-1} + bexp)        ScalarE, reads Y from PSUM
    u_t = h_t + e_t                    VectorE (on-chain), writes u-history
    Y_t = u_t if u_t < thr else 0      VectorE (on-chain), writes PSUM
    h_{t+1} = A*Y_t + J_{t+1}          VectorE prefold (off-chain)
    spk_t = (u_t >= thr)               bulk is_ge over the u-history,
                                       sliced 25x per chunk so each
                                       sub-extract fits the per-step slack

Sharding: batch rows 4k..4k+3 -> core k (4096 neurons/core as [128 x 32]
tiles), serial 2000-step loop per core, no cross-core communication.
Measured step period ~664ns: exp 170ns (PSUM-src) + two DVE ops 204ns +
two semaphore waits ~100ns each (fixed sequencer cost) + issue gaps.
Spike chunks DMA out while the loop continues; J prefetched 2 chunks deep.
Options via env: ADEX_PSUM=1 (default), ADEX_CH chunk size.
"""

import numpy as np

B, T, D = 32, 2000, 1024
N_CORES = 8
BPC = B // N_CORES            # batch rows per core
NPC = BPC * D                 # neurons per core = 4096
W = NPC // 128                # free-dim width = 32


def _build_graph(consts, CH=125, steps=T, use_psum=True):
    import concourse.bass as bass
    import concourse.mybir as mybir

    A, s, bias, thr = consts["A"], consts["s"], consts["bias"], consts["thr"]
    y0 = consts["y0"]
    f32 = mybir.dt.float32
    NCH = steps // CH
    assert steps % CH == 0

    nc = bass.Bass()

    bias_t = nc.alloc_sbuf_tensor("expbias", [128, 1], f32)
    nc.gpsimd.memset(bias_t.ap(), float(bias))
    yinit = nc.alloc_sbuf_tensor("yinit", [128, W], f32)
    nc.gpsimd.memset(yinit.ap(), float(y0))
    nc.all_engine_barrier()

    J_ext = nc.declare_dram_parameter("J", [128, steps, W], f32, isOutput=False)
    spk_ext = nc.declare_dram_parameter("spk", [128, steps, W], f32, isOutput=True)

    with (
        nc.sbuf_tensor([128, 2, CH, W], f32) as jbuf,
        nc.sbuf_tensor([128, 2, CH, W], f32) as uhist,
        nc.sbuf_tensor([128, 2, CH, W], f32) as spkst,
        nc.sbuf_tensor([128, 2, W], f32) as ebuf,
        nc.sbuf_tensor([128, 2, W], f32) as hbuf,
        nc.sbuf_tensor([128, 2, W], f32) as ysb,
        nc.psum_tensor([128, 2, W], f32) as yps,
        nc.semaphore("spk_sem") as spk_sem,
        nc.semaphore("act_sem") as act_sem,
        nc.semaphore("dve_sem") as dve_sem,
        nc.Block() as block,
    ):
        ybuf = yps if use_psum else ysb
        dmaJ_sems = [nc.semaphore(f"dmaJ_sem{p}").__enter__() for p in range(2)]
        dmaS_sems = [nc.semaphore(f"dmaS_sem{p}").__enter__() for p in range(2)]

        def yprev(t):
            # Y_{t-1}: yinit for t=0 else the select output of step t-1
            if t == 0:
                return yinit.ap()
            return ybuf[:, (t - 1) % 2]

        @block.sync
        def _(sync):
            for ci in range(min(2, NCH)):
                sync.dma_start(
                    jbuf[:, ci % 2], J_ext[:, ci * CH:(ci + 1) * CH]
                ).then_inc(dmaJ_sems[ci % 2], 16)
            for ci in range(NCH):
                sync.dma_start(
                    spk_ext[:, ci * CH:(ci + 1) * CH], spkst[:, ci % 2]
                )._wait_ge(spk_sem, 25 * (ci + 1)).then_inc(dmaS_sems[ci % 2], 16)
                if ci + 2 < NCH:
                    sync.dma_start(
                        jbuf[:, ci % 2], J_ext[:, (ci + 2) * CH:(ci + 3) * CH]
                    ).then_inc(dmaJ_sems[ci % 2], 16)

        @block.scalar
        def _(scalar):
            for t in range(steps):
                ins = nc.scalar.activation(
                    ebuf[:, t % 2], yprev(t),
                    mybir.ActivationFunctionType.Exp,
                    bias=bias_t.ap(), scale=float(s),
                ).then_inc(act_sem, 1)
                if t >= 1:
                    ins._wait_ge(dve_sem, t)

        @block.vector
        def _(vector):
            # h_0 prologue: h[0] = A*yinit + J_0
            nc.vector.scalar_tensor_tensor(
                hbuf[:, 0], yinit.ap(), float(A), jbuf[:, 0, 0],
                op0=mybir.AluOpType.mult, op1=mybir.AluOpType.add,
            )._wait_ge(dmaJ_sems[0], 16)
            for t in range(steps):
                ci = t // CH
                # on-chain: u_t = h_t + e_t  (written into the u-history)
                nc.vector.tensor_tensor(
                    uhist[:, ci % 2, t % CH], hbuf[:, t % 2],
                    ebuf[:, t % 2], mybir.AluOpType.add,
                )._wait_ge(act_sem, t + 1)
                # on-chain: Y_t = (u_t < thr) * u_t
                nc.vector.scalar_tensor_tensor(
                    ybuf[:, t % 2],
                    uhist[:, ci % 2, t % CH], float(thr),
                    uhist[:, ci % 2, t % CH],
                    op0=mybir.AluOpType.is_lt, op1=mybir.AluOpType.mult,
                ).then_inc(dve_sem, 1)
                # off-chain: h_{t+1} = A*Y_t + J_{t+1}
                if t + 1 < steps:
                    tn = t + 1
                    cn = tn // CH
                    ins = nc.vector.scalar_tensor_tensor(
                        hbuf[:, tn % 2],
                        ybuf[:, t % 2], float(A),
                        jbuf[:, cn % 2, tn % CH],
                        op0=mybir.AluOpType.mult, op1=mybir.AluOpType.add,
                    )
                    if tn % CH == 0:
                        ins._wait_ge(dmaJ_sems[cn % 2], 16 * (cn // 2 + 1))
                # spike extraction for chunk ci, sliced into E sub-extracts
                # spread across the chunk so the chain never stalls behind
                # one long DVE op: slice k covers steps [k*CH/E, (k+1)*CH/E)
                # and is issued right after its last step completes.
                E = 25
                SL = CH // E
                assert CH % E == 0
                if t % SL == SL - 1:
                    k = (t % CH) // SL
                    ins = nc.vector.tensor_scalar(
                        spkst[:, ci % 2, k * SL:(k + 1) * SL],
                        uhist[:, ci % 2, k * SL:(k + 1) * SL],
                        float(thr), None,
                        mybir.AluOpType.is_ge,
                    ).then_inc(spk_sem, 1)
                    if ci >= 2 and k == 0:
                        ins._wait_ge(dmaS_sems[ci % 2], 16 * ((ci - 2) // 2 + 1))

    return nc


def _derive_consts(params):
    tau_m, E_L, V_T, Delta_T, R, tau_w, a, b, V_reset, V_spike, dt = [
        float(x) for x in params
    ]
    c = dt / tau_m
    return dict(
        A=np.float32(1.0 - c),
        s=np.float32(1.0 / Delta_T),
        bias=np.float32(np.log(c * Delta_T) + (V_reset - V_T) / Delta_T),
        thr=np.float32(V_spike - V_reset),
        y0=np.float32(E_L - V_reset),
        cR=np.float32(c * R),
        Jc=np.float32(c * (E_L - V_reset)),
        a=a, b=b,
    )


def _numpy_fallback(I_seq, params):
    tau_m, E_L, V_T, Delta_T, R, tau_w, a, b, V_reset, V_spike, dt = [
        np.float32(x) for x in params
    ]
    Bs, Ts, Ds = I_seq.shape
    I = I_seq.transpose(1, 0, 2).reshape(Ts, Bs * Ds)
    V = np.full(Bs * Ds, E_L, dtype=np.float32)
    w = np.zeros(Bs * Ds, dtype=np.float32)
    out = np.zeros((Ts, Bs * Ds), dtype=np.float32)
    for t in range(Ts):
        exp_term = Delta_T * np.exp((V - V_T) / Delta_T)
        dV = (-(V - E_L) + exp_term - R * w + R * I[t]) / tau_m
        V = V + dt * dV
        dw = (a * (V - E_L) - w) / tau_w
        w = w + dt * dw
        spk = (V >= V_spike).astype(np.float32)
        V = np.where(spk > 0, V_reset, V)
        w = np.where(spk > 0, w + b, w)
        out[t] = spk
    return out.reshape(Ts, Bs, Ds).transpose(1, 0, 2)


_CACHE = {}


def kernel(I_seq, params):
    I_seq = np.asarray(I_seq, dtype=np.float32)
    params = np.asarray(params, dtype=np.float32)
    consts = _derive_consts(params)
    if consts["a"] != 0.0 or consts["b"] != 0.0:
        return _numpy_fallback(I_seq, params)

    from concourse.bass_utils import run_bass_kernel_spmd

    J = (consts["cR"] * I_seq + consts["Jc"]).astype(np.float32)
    in_maps = []
    for k in range(N_CORES):
        jk = J[BPC * k: BPC * (k + 1)]                       # [4, T, 1024]
        jk = jk.reshape(BPC, T, W, D // W)                   # [4, T, 32, 32]
        jk = np.ascontiguousarray(jk.transpose(0, 2, 1, 3))  # [4, 32, T, 32]
        jk = jk.reshape(128, T, W)
        in_maps.append({"J": jk})

    import os
    CH = int(os.environ.get("ADEX_CH", "125"))
    use_psum = os.environ.get("ADEX_PSUM", "1") == "1"
    key = (np.asarray(params).tobytes(), CH, use_psum)
    if key not in _CACHE:
        _CACHE[key] = _build_graph(consts, CH=CH, use_psum=use_psum)
    nc = _CACHE[key]

    res = run_bass_kernel_spmd(nc, in_maps, core_ids=list(range(N_CORES)))

    out = np.empty((B, T, D), dtype=np.float32)
    for k in range(N_CORES):
        sk = res.results[k]["spk"]                           # [128, T, 32]
        sk = sk.reshape(BPC, W, T, D // W)                   # [4, 32, T, 32]
        sk = sk.transpose(0, 2, 1, 3).reshape(BPC, T, D)     # [4, T, 1024]
        out[BPC * k: BPC * (k + 1)] = sk
    return out
